# revision 32
# baseline (speedup 1.0000x reference)
"""Trainium2 Bass kernel for nn_AttentionHead (softmax over query axis).

Zero-collective design: 8 cores, core pair (2b, 2b+1) both compute batch b
end-to-end (fully redundant); the host reads the even core's output. No
cross-core collectives -> no global-barrier rendezvous, no sensitivity to
the 10-35us per-core launch stagger.

Host ships x already transposed and bf16-cast (xt[e_chunk, e, t]) so the
kernel does zero PE transposes for x. Per core:
  - projections kT/qT/vT [128 D, 2048 t] over the full batch; vT -> v
    natural via 16 PE transposes
  - scores sT[s, t] = kb.T @ qT for the full causal triangle, exp with
    per-key column sums (softmax normalizer is over the QUERY axis);
    diagonal blocks batched 4-per-exp and masked with a host tri tile
  - Z is fully local (all queries present): reciprocal + v scaling
  - z[t, :] = sum_s E[s,t] * (v[s,:]/Z[s]); output streamed in quarters
"""
import sys

for _p in ("/opt/trn_rl_repo",):
    if _p not in sys.path:
        sys.path.append(_p)

import numpy as np
import ml_dtypes

import concourse.bass as bass
import concourse.mybir as mybir
import concourse.tile as tile
from concourse import bacc
from concourse.bass import ds, ts
from concourse.bass_utils import run_bass_kernel_spmd
from concourse.masks import make_identity

BF16 = mybir.dt.bfloat16
F32 = mybir.dt.float32
AF = mybir.ActivationFunctionType
ALU = mybir.AluOpType
AX = mybir.AxisListType

B, T, E, D = 4, 2048, 2048, 128
NE = 16          # E chunks of 128
NTC = 16         # t chunks of 128
NSB = 16         # key blocks of 128
SCALE = 1.0 / np.sqrt(D)
N_CORES = 8


def build_nc():
    nc = bacc.Bacc("TRN2", target_bir_lowering=False, debug=False,
                   num_devices=N_CORES)
    xt = nc.dram_tensor("xt", [NE, 128, T], BF16, kind="ExternalInput")
    wq = nc.dram_tensor("wq", [128, NE, D], BF16, kind="ExternalInput")
    wk = nc.dram_tensor("wk", [128, NE, D], BF16, kind="ExternalInput")
    wv = nc.dram_tensor("wv", [128, NE, D], BF16, kind="ExternalInput")
    masks = nc.dram_tensor("masks", [128, 512], BF16, kind="ExternalInput")
    out = nc.dram_tensor("out", [T, D], F32, kind="ExternalOutput")

    with tile.TileContext(nc) as tc:
        _body(nc, tc, xt, wq, wk, wv, masks, out)
    nc.compile()
    return nc


def _body(nc, tc, xt, wq, wk, wv, masks, out):
    with (
        tc.tile_pool(name="const", bufs=1) as const_pool,
        tc.tile_pool(name="proj", bufs=1) as proj_pool,
        tc.tile_pool(name="escore", bufs=1) as e_pool,
    ):
        # ---- constants (gpsimd SWDGE path; sync ring stays free for xt) ----
        ident = const_pool.tile([128, 128], BF16, name="ident")
        make_identity(nc, ident)
        wq_sb = const_pool.tile([128, NE, D], BF16, name="wq_sb")
        wk_sb = const_pool.tile([128, NE, D], BF16, name="wk_sb")
        wv_sb = const_pool.tile([128, NE, D], BF16, name="wv_sb")
        nc.gpsimd.dma_start(out=wk_sb[:], in_=wk[:])
        nc.gpsimd.dma_start(out=wq_sb[:], in_=wq[:])
        nc.gpsimd.dma_start(out=wv_sb[:], in_=wv[:])
        masks_sb = const_pool.tile([128, 512], BF16, name="masks_sb")
        nc.gpsimd.dma_start(out=masks_sb[:], in_=masks[:])

        # ---- xT load: one contiguous DMA per e-chunk ----
        xt_sb = const_pool.tile([128, NE, T], BF16, name="xt_sb")
        for c in range(NE):
            nc.sync.dma_start(out=xt_sb[:, c, :], in_=xt[c])

        kT_sb = proj_pool.tile([128, T], BF16, name="kT_sb")
        qT_sb = proj_pool.tile([128, T], BF16, name="qT_sb")
        vT_sb = proj_pool.tile([128, T], BF16, name="vT_sb")
        v_nat = proj_pool.tile([128, NSB, D], BF16, name="v_nat")

        # PE warmup spin: get HAM to K=8/8 while the xt DMA streams in.
        zeros = const_pool.tile([128, 128], BF16, name="zeros")
        nc.vector.memset(zeros[:], 0.0)
        with tc.tile_pool(name="wu_psum", bufs=1, space="PSUM") as wu_psum:
            wu = wu_psum.tile([128, 128], F32, tag="wu")
            for _ in range(20):
                nc.tensor.matmul(wu[:], lhsT=zeros[:], rhs=zeros[:],
                                 start=True, stop=True)

        with (
            tc.tile_pool(name="pj_psum", bufs=2, space="PSUM") as pj_psum,
            tc.tile_pool(name="tp_psum", bufs=2, space="PSUM") as tp_psum,
        ):
            for tag, w_sb, dst, eng in (
                ("k_ps", wk_sb, kT_sb, "scalar"),
                ("q_ps", wq_sb, qT_sb, "scalar"),
                ("vt_ps", wv_sb, vT_sb, "vector"),
            ):
                for piece in range(4):
                    ps = pj_psum.tile([128, 512], F32, tag=tag, name=tag)
                    for e in range(NE):
                        nc.tensor.matmul(
                            ps[:], lhsT=w_sb[:, e, :],
                            rhs=xt_sb[:, e, ts(piece, 512)],
                            start=(e == 0), stop=(e == NE - 1),
                        )
                    if eng == "scalar":
                        nc.scalar.copy(out=dst[:, ts(piece, 512)], in_=ps[:])
                    else:
                        nc.vector.tensor_copy(out=dst[:, ts(piece, 512)],
                                              in_=ps[:])
            for grp in range(4):
                tpv = tp_psum.tile([128, 512], BF16, tag="tp", name="tpv")
                for j in range(4):
                    sc = grp * 4 + j
                    nc.tensor.transpose(
                        out=tpv[:, ts(j, 128)],
                        in_=vT_sb[:, ts(sc, 128)],
                        identity=ident[:],
                    )
                nc.vector.tensor_copy(out=v_nat[:, ds(grp * 4, 4), :]
                                      .rearrange("p c d -> p (c d)"),
                                      in_=tpv[:])

        # ---- scores / exp / normalizer (all local: full causal triangle) ----
        stats = const_pool.tile([128, NSB * 5], F32, name="stats")
        zsum_loc = const_pool.tile([128, NSB], F32, name="zsum_loc")
        nc.vector.memset(stats[:], 0.0)
        e_tiles = {}  # (sb, tc) -> AP [128 s, 128 t]
        with (
            tc.tile_pool(name="sc_psum", bufs=2, space="PSUM") as sc_psum,
            tc.tile_pool(name="av_psum", bufs=2, space="PSUM") as av_psum,
        ):
            # diagonal blocks, batched 4 per exp, tri-masked
            for grp in range(4):
                dg = av_psum.tile([128, 512], F32, tag="dgm", name="dg")
                for j in range(4):
                    sb = grp * 4 + j
                    nc.tensor.matmul(
                        dg[:, ts(j, 128)], lhsT=kT_sb[:, ds(sb * 128, 128)],
                        rhs=qT_sb[:, ds(sb * 128, 128)], start=True, stop=True)
                em4 = e_pool.tile([128, 512], BF16, name=f"em4_{grp}",
                                  tag=f"em4_{grp}")
                nc.scalar.activation(out=em4[:], in_=dg[:], func=AF.Exp,
                                     scale=SCALE)
                nc.vector.tensor_tensor(out=em4[:], in0=em4[:],
                                        in1=masks_sb[:], op=ALU.mult)
                for j in range(4):
                    sb = grp * 4 + j
                    nc.vector.reduce_sum(out=stats[:, ds(sb * 5 + 4, 1)],
                                         in_=em4[:, ts(j, 128)], axis=AX.X)
                    e_tiles[(sb, sb)] = em4[:, ts(j, 128)]
            # ---- full blocks per key row sb, with AV for t-chunk (sb-LAG)
            # woven in: per-row zsum -> reciprocal -> v_scale feed a lagged
            # AV accumulation so the PE fills exp-pacing stalls and the
            # kernel ends right after the last exp instead of 14us later.
            recip = const_pool.tile([128, NSB], F32, name="recip")
            v_scaled = proj_pool.tile([128, NSB, D], BF16, name="v_scaled")
            z_all = const_pool.tile([128, NTC, D], F32, name="z_all")
            recip2 = recip  # alias kept for clarity
            LAG = 1

            def emit_av(g):
                zp = av_psum.tile([128, D], F32, tag="zp", name="zp")
                for s2 in range(g + 1):
                    nc.tensor.matmul(
                        zp[:], lhsT=e_tiles[(s2, g)],
                        rhs=v_scaled[:, s2, :],
                        start=(s2 == 0), stop=(s2 == g),
                    )
                if g % 2 == 0:
                    nc.vector.tensor_copy(out=z_all[:, g, :], in_=zp[:])
                else:
                    nc.scalar.copy(out=z_all[:, g, :], in_=zp[:])
                if g >= 14:
                    nc.sync.dma_start(
                        out=out[ds(g * 128, 128), :]
                            .rearrange("(c p) d -> p c d", p=128),
                        in_=z_all[:, ds(g, 1), :],
                    )
                elif g % 2 == 1:
                    h2 = g // 2
                    nc.sync.dma_start(
                        out=out[ds(h2 * 256, 256), :]
                            .rearrange("(c p) d -> p c d", p=128),
                        in_=z_all[:, ds(h2 * 2, 2), :],
                    )

            for sb in range(NSB):
                kb = kT_sb[:, ds(sb * 128, 128)]
                start_tc = sb + 1
                pidx = 0
                while start_tc < NTC:
                    n = min(8, NTC - start_tc)
                    scf = sc_psum.tile([128, 1024], F32, tag="scf",
                                       name="scf")
                    nc.tensor.matmul(
                        scf[:, ds(0, min(n, 4) * 128)], lhsT=kb,
                        rhs=qT_sb[:, ds(start_tc * 128, min(n, 4) * 128)],
                        start=True, stop=True,
                    )
                    if n > 4:
                        nc.tensor.matmul(
                            scf[:, ds(512, (n - 4) * 128)], lhsT=kb,
                            rhs=qT_sb[:, ds((start_tc + 4) * 128,
                                            (n - 4) * 128)],
                            start=True, stop=True,
                        )
                    ef = e_pool.tile([128, n * 128], BF16,
                                     name=f"ef{sb}_{pidx}",
                                     tag=f"ef{sb}_{pidx}")
                    nc.scalar.activation(
                        out=ef[:], in_=scf[:, ds(0, n * 128)], func=AF.Exp,
                        scale=SCALE,
                        accum_out=stats[:, ds(sb * 5 + pidx, 1)],
                    )
                    for j in range(n):
                        e_tiles[(sb, start_tc + j)] = ef[:, ts(j, 128)]
                    start_tc += n
                    pidx += 1
                nc.vector.reduce_sum(out=zsum_loc[:, ds(sb, 1)],
                                     in_=stats[:, ds(sb * 5, 5)],
                                     axis=AX.X)
                nc.vector.reciprocal(out=recip[:, ds(sb, 1)],
                                     in_=zsum_loc[:, ds(sb, 1)])
                nc.vector.tensor_scalar_mul(
                    out=v_scaled[:, sb, :],
                    in0=v_nat[:, sb, :],
                    scalar1=recip[:, ds(sb, 1)],
                )
                if sb >= LAG:
                    emit_av(sb - LAG)
            for g in range(NTC - LAG, NTC):
                emit_av(g)


_NC_CACHE = None


def _get_nc():
    global _NC_CACHE
    if _NC_CACHE is None:
        _NC_CACHE = build_nc()
    return _NC_CACHE


def build_in_maps(x_in, Wq, Wk, Wv):
    """Host-side prep: full-batch transposed bf16 x; pair cores share inputs."""
    x_in = np.asarray(x_in, dtype=np.float32)
    ws = {}
    for name, W in (("wq", Wq), ("wk", Wk), ("wv", Wv)):
        W = np.asarray(W, dtype=np.float32)
        ws[name] = np.ascontiguousarray(
            W.reshape(NE, 128, D).transpose(1, 0, 2)
        ).astype(ml_dtypes.bfloat16)
    tri = (np.arange(128)[None, :] >= np.arange(128)[:, None]).astype(np.float32)
    masks = np.ascontiguousarray(np.tile(tri, (1, 4))).astype(ml_dtypes.bfloat16)
    per_batch = []
    for b in range(B):
        xt = np.ascontiguousarray(x_in[b].T).reshape(NE, 128, T)
        per_batch.append(xt.astype(ml_dtypes.bfloat16))
    in_maps = []
    for c in range(N_CORES):
        in_maps.append({
            "xt": per_batch[c // 2],
            "wq": ws["wq"], "wk": ws["wk"], "wv": ws["wv"],
            "masks": masks,
        })
    return in_maps


def kernel(x_in, Wq, Wk, Wv):
    nc = _get_nc()
    in_maps = build_in_maps(x_in, Wq, Wk, Wv)
    res = run_bass_kernel_spmd(nc, in_maps, core_ids=list(range(N_CORES)))
    out = np.empty((B, T, D), np.float32)
    for b in range(B):
        out[b] = res.results[2 * b]["out"]
    return out


# revision 33
# speedup vs baseline: 1.0051x; 1.0051x over previous
"""Trainium2 Bass kernel for nn_AttentionHead (softmax over query axis).

Zero-collective design: 8 cores, core pair (2b, 2b+1) both compute batch b
end-to-end (fully redundant); the host reads the even core's output. No
cross-core collectives -> no global-barrier rendezvous, no sensitivity to
the 10-35us per-core launch stagger.

Host ships x already transposed and bf16-cast (xt[e_chunk, e, t]) so the
kernel does zero PE transposes for x. Per core:
  - projections kT/qT/vT [128 D, 2048 t] over the full batch; vT -> v
    natural via 16 PE transposes
  - scores sT[s, t] = kb.T @ qT for the full causal triangle, exp with
    per-key column sums (softmax normalizer is over the QUERY axis);
    diagonal blocks batched 4-per-exp and masked with a host tri tile
  - Z is fully local (all queries present): reciprocal + v scaling
  - z[t, :] = sum_s E[s,t] * (v[s,:]/Z[s]); output streamed in quarters
"""
import sys

for _p in ("/opt/trn_rl_repo",):
    if _p not in sys.path:
        sys.path.append(_p)

import numpy as np
import ml_dtypes

import concourse.bass as bass
import concourse.mybir as mybir
import concourse.tile as tile
from concourse import bacc
from concourse.bass import ds, ts
from concourse.bass_utils import run_bass_kernel_spmd
from concourse.masks import make_identity

BF16 = mybir.dt.bfloat16
F32 = mybir.dt.float32
AF = mybir.ActivationFunctionType
ALU = mybir.AluOpType
AX = mybir.AxisListType

B, T, E, D = 4, 2048, 2048, 128
NE = 16          # E chunks of 128
NTC = 16         # t chunks of 128
NSB = 16         # key blocks of 128
SCALE = 1.0 / np.sqrt(D)
N_CORES = 8


def build_nc():
    nc = bacc.Bacc("TRN2", target_bir_lowering=False, debug=False,
                   num_devices=N_CORES)
    xt = nc.dram_tensor("xt", [NE, 128, T], BF16, kind="ExternalInput")
    wq = nc.dram_tensor("wq", [128, NE, D], BF16, kind="ExternalInput")
    wk = nc.dram_tensor("wk", [128, NE, D], BF16, kind="ExternalInput")
    wv = nc.dram_tensor("wv", [128, NE, D], BF16, kind="ExternalInput")
    masks = nc.dram_tensor("masks", [128, 512], BF16, kind="ExternalInput")
    out = nc.dram_tensor("out", [T, D], F32, kind="ExternalOutput")

    with tile.TileContext(nc) as tc:
        _body(nc, tc, xt, wq, wk, wv, masks, out)
    nc.compile()
    return nc


def _body(nc, tc, xt, wq, wk, wv, masks, out):
    with (
        tc.tile_pool(name="const", bufs=1) as const_pool,
        tc.tile_pool(name="proj", bufs=1) as proj_pool,
        tc.tile_pool(name="escore", bufs=1) as e_pool,
    ):
        # ---- constants (gpsimd SWDGE path; sync ring stays free for xt) ----
        ident = const_pool.tile([128, 128], BF16, name="ident")
        make_identity(nc, ident)
        wq_sb = const_pool.tile([128, NE, D], BF16, name="wq_sb")
        wk_sb = const_pool.tile([128, NE, D], BF16, name="wk_sb")
        wv_sb = const_pool.tile([128, NE, D], BF16, name="wv_sb")
        nc.gpsimd.dma_start(out=wk_sb[:], in_=wk[:])
        nc.gpsimd.dma_start(out=wq_sb[:], in_=wq[:])
        nc.gpsimd.dma_start(out=wv_sb[:], in_=wv[:])
        masks_sb = const_pool.tile([128, 512], BF16, name="masks_sb")
        nc.gpsimd.dma_start(out=masks_sb[:], in_=masks[:])

        # ---- xT load: one contiguous DMA per e-chunk ----
        xt_sb = const_pool.tile([128, NE, T], BF16, name="xt_sb")
        for c in range(NE):
            nc.sync.dma_start(out=xt_sb[:, c, :], in_=xt[c])

        kT_sb = proj_pool.tile([128, T], BF16, name="kT_sb")
        qT_sb = proj_pool.tile([128, T], BF16, name="qT_sb")
        vT_sb = proj_pool.tile([128, T], BF16, name="vT_sb")
        v_nat = proj_pool.tile([128, NSB, D], BF16, name="v_nat")

        # PE warmup spin: get HAM to K=8/8 while the xt DMA streams in.
        zeros = const_pool.tile([128, 128], BF16, name="zeros")
        nc.vector.memset(zeros[:], 0.0)
        with tc.tile_pool(name="wu_psum", bufs=1, space="PSUM") as wu_psum:
            wu = wu_psum.tile([128, 128], F32, tag="wu")
            for _ in range(20):
                nc.tensor.matmul(wu[:], lhsT=zeros[:], rhs=zeros[:],
                                 start=True, stop=True)

        with (
            tc.tile_pool(name="pj_psum", bufs=2, space="PSUM") as pj_psum,
            tc.tile_pool(name="tp_psum", bufs=2, space="PSUM") as tp_psum,
        ):
            for tag, w_sb, dst, eng in (
                ("k_ps", wk_sb, kT_sb, "scalar"),
                ("q_ps", wq_sb, qT_sb, "scalar"),
                ("vt_ps", wv_sb, vT_sb, "vector"),
            ):
                for piece in range(4):
                    ps = pj_psum.tile([128, 512], F32, tag=tag, name=tag)
                    for e in range(NE):
                        nc.tensor.matmul(
                            ps[:], lhsT=w_sb[:, e, :],
                            rhs=xt_sb[:, e, ts(piece, 512)],
                            start=(e == 0), stop=(e == NE - 1),
                        )
                    if eng == "scalar":
                        nc.scalar.copy(out=dst[:, ts(piece, 512)], in_=ps[:])
                    else:
                        nc.vector.tensor_copy(out=dst[:, ts(piece, 512)],
                                              in_=ps[:])
            for grp in range(4):
                tpv = tp_psum.tile([128, 512], BF16, tag="tp", name="tpv")
                for j in range(4):
                    sc = grp * 4 + j
                    nc.tensor.transpose(
                        out=tpv[:, ts(j, 128)],
                        in_=vT_sb[:, ts(sc, 128)],
                        identity=ident[:],
                    )
                nc.vector.tensor_copy(out=v_nat[:, ds(grp * 4, 4), :]
                                      .rearrange("p c d -> p (c d)"),
                                      in_=tpv[:])

        # ---- scores / exp / normalizer (all local: full causal triangle) ----
        stats = const_pool.tile([128, NSB * 5], F32, name="stats")
        zsum_loc = const_pool.tile([128, NSB], F32, name="zsum_loc")
        nc.vector.memset(stats[:], 0.0)
        e_tiles = {}  # (sb, tc) -> AP [128 s, 128 t]
        with (
            tc.tile_pool(name="sc_psum", bufs=2, space="PSUM") as sc_psum,
            tc.tile_pool(name="av_psum", bufs=2, space="PSUM") as av_psum,
        ):
            # diagonal blocks, batched 4 per exp, tri-masked
            for grp in range(4):
                dg = av_psum.tile([128, 512], F32, tag="dgm", name="dg")
                for j in range(4):
                    sb = grp * 4 + j
                    nc.tensor.matmul(
                        dg[:, ts(j, 128)], lhsT=kT_sb[:, ds(sb * 128, 128)],
                        rhs=qT_sb[:, ds(sb * 128, 128)], start=True, stop=True)
                em4 = e_pool.tile([128, 512], BF16, name=f"em4_{grp}",
                                  tag=f"em4_{grp}")
                nc.scalar.activation(out=em4[:], in_=dg[:], func=AF.Exp,
                                     scale=SCALE)
                nc.vector.tensor_tensor(out=em4[:], in0=em4[:],
                                        in1=masks_sb[:], op=ALU.mult)
                for j in range(4):
                    sb = grp * 4 + j
                    nc.vector.reduce_sum(out=stats[:, ds(sb * 5 + 4, 1)],
                                         in_=em4[:, ts(j, 128)], axis=AX.X)
                    e_tiles[(sb, sb)] = em4[:, ts(j, 128)]
            # ---- full blocks per key row sb, with AV for t-chunk (sb-LAG)
            # woven in: per-row zsum -> reciprocal -> v_scale feed a lagged
            # AV accumulation so the PE fills exp-pacing stalls and the
            # kernel ends right after the last exp instead of 14us later.
            recip = const_pool.tile([128, NSB], F32, name="recip")
            v_scaled = proj_pool.tile([128, NSB, D], BF16, name="v_scaled")
            z_all = const_pool.tile([128, NTC, D], F32, name="z_all")
            recip2 = recip  # alias kept for clarity
            LAG = 2

            def emit_av(g):
                zp = av_psum.tile([128, D], F32, tag="zp", name="zp")
                for s2 in range(g + 1):
                    nc.tensor.matmul(
                        zp[:], lhsT=e_tiles[(s2, g)],
                        rhs=v_scaled[:, s2, :],
                        start=(s2 == 0), stop=(s2 == g),
                    )
                if g % 2 == 0:
                    nc.vector.tensor_copy(out=z_all[:, g, :], in_=zp[:])
                else:
                    nc.scalar.copy(out=z_all[:, g, :], in_=zp[:])
                if g % 2 == 1:
                    h2 = g // 2
                    nc.sync.dma_start(
                        out=out[ds(h2 * 256, 256), :]
                            .rearrange("(c p) d -> p c d", p=128),
                        in_=z_all[:, ds(h2 * 2, 2), :],
                    )

            for sb in range(NSB):
                kb = kT_sb[:, ds(sb * 128, 128)]
                start_tc = sb + 1
                pidx = 0
                while start_tc < NTC:
                    n = min(8, NTC - start_tc)
                    scf = sc_psum.tile([128, 1024], F32, tag="scf",
                                       name="scf")
                    nc.tensor.matmul(
                        scf[:, ds(0, min(n, 4) * 128)], lhsT=kb,
                        rhs=qT_sb[:, ds(start_tc * 128, min(n, 4) * 128)],
                        start=True, stop=True,
                    )
                    if n > 4:
                        nc.tensor.matmul(
                            scf[:, ds(512, (n - 4) * 128)], lhsT=kb,
                            rhs=qT_sb[:, ds((start_tc + 4) * 128,
                                            (n - 4) * 128)],
                            start=True, stop=True,
                        )
                    ef = e_pool.tile([128, n * 128], BF16,
                                     name=f"ef{sb}_{pidx}",
                                     tag=f"ef{sb}_{pidx}")
                    nc.scalar.activation(
                        out=ef[:], in_=scf[:, ds(0, n * 128)], func=AF.Exp,
                        scale=SCALE,
                        accum_out=stats[:, ds(sb * 5 + pidx, 1)],
                    )
                    for j in range(n):
                        e_tiles[(sb, start_tc + j)] = ef[:, ts(j, 128)]
                    start_tc += n
                    pidx += 1
                nc.vector.reduce_sum(out=zsum_loc[:, ds(sb, 1)],
                                     in_=stats[:, ds(sb * 5, 5)],
                                     axis=AX.X)
                nc.vector.reciprocal(out=recip[:, ds(sb, 1)],
                                     in_=zsum_loc[:, ds(sb, 1)])
                nc.vector.tensor_scalar_mul(
                    out=v_scaled[:, sb, :],
                    in0=v_nat[:, sb, :],
                    scalar1=recip[:, ds(sb, 1)],
                )
                if sb >= LAG:
                    emit_av(sb - LAG)
            for g in range(NTC - LAG, NTC):
                emit_av(g)


_NC_CACHE = None


def _get_nc():
    global _NC_CACHE
    if _NC_CACHE is None:
        _NC_CACHE = build_nc()
    return _NC_CACHE


def build_in_maps(x_in, Wq, Wk, Wv):
    """Host-side prep: full-batch transposed bf16 x; pair cores share inputs."""
    x_in = np.asarray(x_in, dtype=np.float32)
    ws = {}
    for name, W in (("wq", Wq), ("wk", Wk), ("wv", Wv)):
        W = np.asarray(W, dtype=np.float32)
        ws[name] = np.ascontiguousarray(
            W.reshape(NE, 128, D).transpose(1, 0, 2)
        ).astype(ml_dtypes.bfloat16)
    tri = (np.arange(128)[None, :] >= np.arange(128)[:, None]).astype(np.float32)
    masks = np.ascontiguousarray(np.tile(tri, (1, 4))).astype(ml_dtypes.bfloat16)
    per_batch = []
    for b in range(B):
        xt = np.ascontiguousarray(x_in[b].T).reshape(NE, 128, T)
        per_batch.append(xt.astype(ml_dtypes.bfloat16))
    in_maps = []
    for c in range(N_CORES):
        in_maps.append({
            "xt": per_batch[c // 2],
            "wq": ws["wq"], "wk": ws["wk"], "wv": ws["wv"],
            "masks": masks,
        })
    return in_maps


def kernel(x_in, Wq, Wk, Wv):
    nc = _get_nc()
    in_maps = build_in_maps(x_in, Wq, Wk, Wv)
    res = run_bass_kernel_spmd(nc, in_maps, core_ids=list(range(N_CORES)))
    out = np.empty((B, T, D), np.float32)
    for b in range(B):
        out[b] = res.results[2 * b]["out"]
    return out
